# revision 7
# baseline (speedup 1.0000x reference)
import numpy as np
import ml_dtypes

bf16 = ml_dtypes.bfloat16

H = 12
HS = 64
ALL = H * HS          # 768
P = 128
B = 2
S = 1024
C = 64                # output channels (W_out cols)
SCALING = HS ** 0.25  # 2.8284...
S_CORE = 256          # s-rows per core
NSLAB = S_CORE // 8   # 32 slabs of 8 s-rows
NCORES = 8

IN_NAMES = ["mrg"]
# merged bf16 blob: p1T | p1Tq | wqk | relPad2 | wfull | b2d (row-major)
SEC_SIZES = [ALL * S, ALL * S_CORE, ALL * 2 * ALL, 128 * 2048, 128 * 512, 128 * 12]
SEC_OFFS = [0]
for _s in SEC_SIZES[:-1]:
    SEC_OFFS.append(SEC_OFFS[-1] + _s)
MRG_LEN = SEC_OFFS[-1] + SEC_SIZES[-1]

_COMPILED = None


def _build_nc(nslab=NSLAB):
    import concourse.bacc as bacc
    import concourse.mybir as mybir
    from concourse.tile import TileContext

    dt = mybir.dt
    AF = mybir.ActivationFunctionType

    nc = bacc.Bacc(enable_partition_id=False)

    mrg = nc.dram_tensor("mrg", [MRG_LEN], dt.bfloat16, kind="ExternalInput")
    b2d = mrg[SEC_OFFS[5] : SEC_OFFS[5] + SEC_SIZES[5]]
    p1T = mrg[SEC_OFFS[0] : SEC_OFFS[0] + SEC_SIZES[0]]
    p1Tq = mrg[SEC_OFFS[1] : SEC_OFFS[1] + SEC_SIZES[1]]
    wqk = mrg[SEC_OFFS[2] : SEC_OFFS[2] + SEC_SIZES[2]]
    relPad2 = mrg[SEC_OFFS[3] : SEC_OFFS[3] + SEC_SIZES[3]]
    wfull = mrg[SEC_OFFS[4] : SEC_OFFS[4] + SEC_SIZES[4]]

    # blocked output: [slab, t_within_chunk, t_chunk, s_within_slab, c]
    out = nc.dram_tensor("out", [NSLAB, 128, 8, 8, C], dt.bfloat16, kind="ExternalOutput")

    inv_s = float(1.0 / SCALING)

    with TileContext(nc) as tc:
        with (
            tc.tile_pool(name="const", bufs=1) as cpool,
            tc.tile_pool(name="persist", bufs=1) as ppool,
            tc.tile_pool(name="slab", bufs=2) as spool,
            tc.tile_pool(name="outp", bufs=3) as opool,
        ):
            # ---- constant loads ----
            wqk_t = cpool.tile([128, 6, 2 * ALL], dt.bfloat16)
            p1T_t = cpool.tile([128, 6, S], dt.bfloat16)
            p1Tq_t = cpool.tile([128, 6, S_CORE], dt.bfloat16)
            wqk_v = wqk.rearrange("(a p f) -> p a f", p=128, f=2 * ALL)
            p1T_v = p1T.rearrange("(a p s) -> p a s", p=128, s=S)
            p1Tq_v = p1Tq.rearrange("(a p s) -> p a s", p=128, s=S_CORE)
            for ca in range(6):
                nc.sync.dma_start(out=wqk_t[:, ca, :], in_=wqk_v[:, ca, :])
                nc.sync.dma_start(out=p1T_t[:, ca, :], in_=p1T_v[:, ca, :])
                nc.sync.dma_start(out=p1Tq_t[:, ca, :], in_=p1Tq_v[:, ca, :])
            b2d_bf = cpool.tile([128, 12], dt.bfloat16)
            nc.sync.dma_start(out=b2d_bf[:], in_=b2d.rearrange("(p u) -> p u", u=12))
            b2d_t = cpool.tile([128, 12], dt.float32)
            nc.vector.tensor_copy(b2d_t[:], b2d_bf[:])
            rel_t = cpool.tile([128, 2048], dt.bfloat16)
            nc.sync.dma_start(out=rel_t[:], in_=relPad2.rearrange("(p u) -> p u", u=2048))
            wfull_t = cpool.tile([128, 512], dt.bfloat16)
            nc.sync.dma_start(out=wfull_t[:], in_=wfull.rearrange("(p u) -> p u", u=512))

            # ---- projections: kT (all S), qT (own S_CORE) ----
            kT_t = ppool.tile([128, 6, S], dt.bfloat16)    # rows f = 768 + cf*128 + p
            qT_t = ppool.tile([128, 6, S_CORE], dt.bfloat16)
            with tc.tile_pool(name="ps_proj", bufs=2, space="PSUM") as pj_pool:
                for cf in range(6):
                    for th in range(2):
                        pj = pj_pool.tile([128, 512], dt.float32)
                        for ca in range(6):
                            nc.tensor.matmul(
                                pj[:],
                                lhsT=wqk_t[:, ca, ALL + cf * 128 : ALL + cf * 128 + 128],
                                rhs=p1T_t[:, ca, th * 512 : th * 512 + 512],
                                start=(ca == 0),
                                stop=(ca == 5),
                            )
                        nc.scalar.activation(
                            kT_t[:, cf, th * 512 : th * 512 + 512],
                            pj[:],
                            AF.Identity,
                            bias=b2d_t[:, 6 + cf : 7 + cf],
                            scale=inv_s,
                        )
                for cf in range(6):
                    pj = pj_pool.tile([128, 512], dt.float32)
                    for ca in range(6):
                        nc.tensor.matmul(
                            pj[:, 0:S_CORE],
                            lhsT=wqk_t[:, ca, cf * 128 : cf * 128 + 128],
                            rhs=p1Tq_t[:, ca, :],
                            start=(ca == 0),
                            stop=(ca == 5),
                        )
                    nc.scalar.activation(
                        qT_t[:, cf, :],
                        pj[:, 0:S_CORE],
                        AF.Identity,
                        bias=b2d_t[:, cf : cf + 1],
                        scale=inv_s,
                    )

            # ---- bulk lhsT staging for all slabs ----
            # scores lhsT: qbd_all[p=(p2,d), c6, col = 128*g + 16*a + (2*c6+p2)]
            #   = q[h=2*c6+p2, d, s = 8*g + a]
            qbd_all = ppool.tile([128, 6, 128 * NSLAB], dt.bfloat16)
            qv = qbd_all.rearrange("p k (g a b) -> p k g a b", a=8, b=16)
            nc.vector.memset(qbd_all.rearrange("p k x -> p (k x)"), 0.0)
            for c6 in range(6):
                for p2 in range(2):
                    r0 = 64 * p2
                    nc.vector.tensor_copy(
                        qv[r0 : r0 + 64, c6, :, :, 2 * c6 + p2],
                        qT_t[r0 : r0 + 64, c6, :].rearrange("p (g a) -> p g a", a=8),
                    )

            # bias lhsT: qbp_all[p=(v,d), col = 512*g + 128*j + 16*(2*j+v) + h]
            #   = q[h, d, s = 8*g + 2*j + v]
            # staged via qTds[p=(p2,d), col = 6*s + cf] = q[h=2*cf+p2, d, s]
            qTds = ppool.tile([128, 6 * S_CORE], dt.bfloat16)
            qTds_v = qTds.rearrange("p (s k) -> p s k", k=6)
            for cf in range(6):
                nc.scalar.activation(qTds_v[:, :, cf], qT_t[:, cf, :], AF.Copy)

            qbp_all = ppool.tile([128, 512 * NSLAB], dt.bfloat16)
            nc.gpsimd.memset(qbp_all[:], 0.0)
            qbp_v = qbp_all.rearrange("p (g r two) -> p g r two", r=256, two=2)
            qTds_s = qTds.rearrange("p (g r k) -> p g r k", r=8, k=6)
            for v in range(2):
                for j in range(4):
                    c0h = (160 * j + 16 * v) // 2
                    for p2 in range(2):
                        nc.vector.tensor_copy(
                            qbp_v[64 * v : 64 * v + 64, :, c0h : c0h + 6, p2],
                            qTds_s[64 * p2 : 64 * p2 + 64, :, 2 * j + v, :],
                        )

            slab_pools = (
                tc.tile_pool(name="ps_a", bufs=2, space="PSUM"),
                tc.tile_pool(name="ps_o", bufs=3, space="PSUM"),
            )
            pa_pool = slab_pools[0].__enter__()
            po_pool = slab_pools[1].__enter__()

            # ---- per-slab pipeline ----
            def slab(g):
                s0 = 8 * g
                a_t = spool.tile([128, S], dt.bfloat16, tag="a")
                pa0 = pa_pool.tile([128, 512], dt.float32, name="pa0", tag="pa0")
                pa1 = pa_pool.tile([128, 512], dt.float32, name="pa1", tag="pa1")
                pas = (pa0, pa1)
                # scores: accumulate over 6 head-pair chunks of W_qk features
                for c6 in range(6):
                    for th in range(2):
                        nc.tensor.matmul(
                            pas[th][:],
                            lhsT=qbd_all[:, c6, 128 * g : 128 * g + 128],
                            rhs=kT_t[:, c6, th * 512 : th * 512 + 512],
                            start=(c6 == 0),
                            stop=False,
                        )
                # rel-pos bias: 4 band-pair matmuls against shifted views of
                # the edge-clamped rel table (shift = compile-time col offset)
                for j in range(4):
                    c0 = 1024 - (s0 + 2 * j)
                    for th in range(2):
                        nc.tensor.matmul(
                            pas[th][:],
                            lhsT=qbp_all[:, 512 * g + 128 * j : 512 * g + 128 * j + 128],
                            rhs=rel_t[:, c0 + th * 512 : c0 + th * 512 + 512],
                            start=False,
                            stop=(j == 3),
                        )
                nc.scalar.activation(a_t[:, 0:512], pa0[:], AF.Copy)
                nc.vector.tensor_copy(a_t[:, 512:1024], pa1[:])

                # final W_out contraction with A as stationary weights
                so = opool.tile([128, 8, 8, 64], dt.bfloat16, tag="so")
                for tc_i in range(8):
                    po = po_pool.tile([128, 512], dt.float32)
                    nc.tensor.matmul(
                        po[:],
                        lhsT=a_t[:, tc_i * 128 : tc_i * 128 + 128],
                        rhs=wfull_t[:],
                        start=True,
                        stop=True,
                    )
                    so_dst = so[:, tc_i, :, :].rearrange("p a b -> p (a b)")
                    if tc_i % 2 == 0:
                        nc.scalar.activation(so_dst, po[:], AF.Copy)
                    else:
                        nc.vector.tensor_copy(so_dst, po[:])
                nc.sync.dma_start(out=out[g], in_=so[:])

            for g in range(nslab):
                slab(g)

            for cm in reversed(slab_pools):
                cm.__exit__(None, None, None)

    nc.finalize()
    return nc


def _host_prep(p1, W_qk, b_qk, rel_emb, W_out, b_out):
    wqk_bf = W_qk.astype(bf16)
    b2d = np.ascontiguousarray((b_qk / SCALING).reshape(12, 128).T.astype(np.float32))
    rel_scaled = (rel_emb / SCALING).astype(np.float32)  # [258, 64]
    wfull = np.zeros((128, 512), np.float32)
    for grp in range(8):
        wfull[grp * 16 : grp * 16 + H, grp * 64 : grp * 64 + 64] = W_out
    wfull = wfull.astype(bf16)

    p1T = [np.ascontiguousarray(p1[b].T).astype(bf16) for b in range(B)]

    in_maps = []
    u = np.arange(2048)
    for core in range(NCORES):
        b = core // 4
        s_off = (core % 4) * S_CORE
        p1Tq = np.ascontiguousarray(p1T[b][:, s_off : s_off + S_CORE])

        # relPad2[64*v + d, u] = rel_scaled[row(clip(u - s_off - 1024 - v))][d]
        # (per-core origin baked in so one SPMD NEFF serves all cores)
        relPad2 = np.empty((128, 2048), np.float32)
        for v in range(2):
            e = np.clip(u - s_off - 1024 - v, -127, 127)
            rows = np.where(e >= 0, e, e + 256)
            relPad2[64 * v : 64 * v + 64, :] = rel_scaled[rows, :].T
        relPad2 = relPad2.astype(bf16)

        mrg = np.concatenate([
            np.ravel(p1T[b]), np.ravel(p1Tq), np.ravel(wqk_bf),
            np.ravel(relPad2), np.ravel(wfull), np.ravel(b2d.astype(bf16)),
        ])
        in_maps.append({"mrg": mrg})
    return in_maps


class _Runner:
    """Holds the AOT fast-dispatch executable (C++ pjit fast path, no
    donated output: the kernel writes every out element, so PJRT's
    uninitialized result allocation is safe)."""

    def __init__(self):
        import jax
        from jax.experimental.shard_map import shard_map
        from jax.sharding import Mesh, NamedSharding, PartitionSpec

        from concourse import bass2jax
        from concourse.bass2jax import _bass_exec_p, install_neuronx_cc_hook

        try:
            from concourse.bass2jax import fast_dispatch_compile
        except ImportError:
            fast_dispatch_compile = None

        install_neuronx_cc_hook()
        nc = _build_nc()
        self.nc = nc

        out_aval = jax.core.ShapedArray((NSLAB, 128, 8, 8, C), np.dtype(bf16))
        pname = nc.partition_id_tensor.name if nc.partition_id_tensor else None
        all_in = list(IN_NAMES) + ([pname] if pname else [])

        def _body(*args):
            operands = list(args)
            if pname:
                operands.append(bass2jax.partition_id_tensor())
            outs = _bass_exec_p.bind(
                *operands,
                out_avals=(out_aval,),
                in_names=tuple(all_in),
                out_names=("out",),
                lowering_input_output_aliases=(),
                sim_require_finite=True,
                sim_require_nnan=True,
                nc=nc,
            )
            return tuple(outs)

        devices = jax.devices()[:NCORES]
        self.mesh = Mesh(np.asarray(devices), ("core",))
        self.sharding = NamedSharding(self.mesh, PartitionSpec("core"))
        in_shapes = {"mrg": (MRG_LEN,)}
        in_dtypes = {"mrg": np.dtype(bf16)}
        arg_structs = [
            jax.ShapeDtypeStruct(
                (NCORES * in_shapes[n][0],) + in_shapes[n][1:],
                in_dtypes[n], sharding=self.sharding)
            for n in IN_NAMES
        ]
        def _compile():
            return jax.jit(
                shard_map(
                    _body, mesh=self.mesh,
                    in_specs=(PartitionSpec("core"),) * len(IN_NAMES),
                    out_specs=(PartitionSpec("core"),), check_rep=False),
                keep_unused=True,
            ).lower(*arg_structs).compile()

        if fast_dispatch_compile is not None:
            self.fast = fast_dispatch_compile(_compile)
        else:
            self.fast = _compile()
        self._jax = jax

    def device_inputs(self, in_maps):
        jax = self._jax
        concat_in = [
            np.concatenate([np.asarray(m[name]) for m in in_maps], axis=0)
            for name in IN_NAMES
        ]
        return [jax.device_put(a, self.sharding) for a in concat_in]

    def run(self, in_maps, out_f32):
        """Execute and write fp32 results directly into out_f32 [B,S,S,C]."""
        dev_in = self.device_inputs(in_maps)
        out_arr = self.fast(*dev_in)[0]

        shards = sorted(out_arr.addressable_shards, key=lambda sh: sh.index[0].start)

        def convert(core, raw):
            b = core // 4
            s_off = (core % 4) * S_CORE
            # fast bf16 -> f32 upcast, then unblock (g,t,u,s,c) -> (s_glob, t_glob, c)
            u = raw.view(np.uint16).astype(np.uint32) << 16
            f = u.view(np.float32)
            out_f32[b, s_off : s_off + S_CORE] = (
                f.transpose(0, 3, 2, 1, 4).reshape(S_CORE, S, C)
            )

        # downloads are serial (tunnel-bound); hide the upcast/detranspose
        # behind the next shard's download via a single worker thread
        from concurrent.futures import ThreadPoolExecutor

        with ThreadPoolExecutor(1) as ex:
            futs = []
            for core, sh in enumerate(shards):
                raw = np.asarray(sh.data)  # network-bound fetch
                futs.append(ex.submit(convert, core, raw))
            for fu in futs:
                fu.result()


def kernel(p0, p1, p2, W_qk, b_qk, rel_emb, W_out, b_out):
    global _COMPILED

    p1 = np.asarray(p1, np.float32)
    W_qk = np.asarray(W_qk, np.float32)
    b_qk = np.asarray(b_qk, np.float32)
    rel_emb = np.asarray(rel_emb, np.float32)
    W_out = np.asarray(W_out, np.float32)
    b_out = np.asarray(b_out, np.float32)

    if _COMPILED is None:
        _COMPILED = _Runner()
    run = _COMPILED

    in_maps = _host_prep(p1, W_qk, b_qk, rel_emb, W_out, b_out)
    full = np.empty((B, S, S, C), np.float32)
    run.run(in_maps, full)
    if np.any(b_out):
        full += np.asarray(b_out, np.float32)
    return full
